# revision 11
# baseline (speedup 1.0000x reference)
"""Single-head causal attention (B=8, T=2048, D=1024, H=64) on 8 TRN2 NeuronCores.

Sharding: data-parallel over batch B — core b computes attention for x[b].

Per-core algorithm (all matmuls bf16 with f32 PSUM accumulation):
  1. x [T, D] f32 is cast to bf16 during the SWDGE DMA load, then DMA-xbar
     transposed (bf16) into xT [D, T] in SBUF (D on partitions, 8 chunks of 128).
  2. Projections computed transposed: qT/kT/vT [H=64, T] = W.T @ x.T with the
     weight chunk as the stationary operand (PSUM accumulate over 8 D-chunks).
  3. vT is DMA-transposed back to v tiles [128, H] and augmented with a ones
     column -> v_aug [128, H+1]; the PV matmul then yields row-sums for free.
  4. Scores are computed TRANSPOSED (sT[k, q] = k @ qT, K=64 contraction) so
     the exp'd tile is directly the stationary operand of the PV matmul --
     no per-tile transpose of the probabilities is ever needed.
     Softmax skips the max-subtraction: scores*0.125 are ~N(0,1) (|s|<~7), so
     exp is numerically safe in f32/bf16. The 0.125 scale is folded into the
     ACT exp instruction. Causality: only kj<=qi blocks are computed; the
     diagonal block is masked by a 0/1 upper-triangular multiply AFTER exp.
  5. out[q, :] = (sum_k p[k,q]*v_aug[k, :]) accumulated over kj blocks in PSUM;
     final division by the row-sum (column H) happens at PSUM evacuation.
"""

import numpy as np

B, T, D, H = 8, 2048, 1024, 64
P = 128          # partition tile
NT = T // P      # 16 T-tiles
ND = D // P      # 8 D-chunks
NCORES = 8
SCALE = float(H) ** -0.5  # 0.125
SCORE_CHUNK = 1024       # PSUM score tile free size (2 banks)

_CACHE = {}


def _build_nc():
    import concourse.bass as bass
    import concourse.tile as tile
    from concourse import bacc, mybir

    # Bacc (not Bass): its compile() runs the TRN2 sync-wait splitting pass
    # (walrus rejects multi-wait Drain instructions otherwise).
    nc = bacc.Bacc(
        "TRN2", target_bir_lowering=False, debug=False, num_devices=NCORES
    )
    f32 = mybir.dt.float32
    bf16 = mybir.dt.bfloat16

    x_d = nc.declare_dram_parameter("x", [T, D], f32, isOutput=False)
    wq_d = nc.declare_dram_parameter("wq", [D, H], f32, isOutput=False)
    wk_d = nc.declare_dram_parameter("wk", [D, H], f32, isOutput=False)
    wv_d = nc.declare_dram_parameter("wv", [D, H], f32, isOutput=False)
    mask_d = nc.declare_dram_parameter("mask", [P, P], bf16, isOutput=False)
    out_d = nc.declare_dram_parameter("out", [T, H], f32, isOutput=True)

    ts = bass.ts
    Exp = mybir.ActivationFunctionType.Exp

    with tile.TileContext(nc) as tc:
        with (
            tc.tile_pool(name="consts", bufs=1) as consts,
            tc.tile_pool(name="bigs", bufs=1) as bigs,
            tc.tile_pool(name="xstage", bufs=3) as xstage,
            tc.tile_pool(name="evac", bufs=3) as evac,
        ):
            # ---- constants ----
            wq_sb = consts.tile([P, ND, H], bf16)
            wk_sb = consts.tile([P, ND, H], bf16)
            wv_sb = consts.tile([P, ND, H], bf16)
            mask_sb = consts.tile([P, P], bf16)
            # SWDGE cast-DMA: f32 DRAM -> bf16 SBUF, D-chunked on partitions
            nc.gpsimd.dma_start(wq_sb[:], wq_d[:].rearrange("(dc p) h -> p dc h", p=P))
            nc.gpsimd.dma_start(wk_sb[:], wk_d[:].rearrange("(dc p) h -> p dc h", p=P))
            nc.gpsimd.dma_start(wv_sb[:], wv_d[:].rearrange("(dc p) h -> p dc h", p=P))
            nc.sync.dma_start(mask_sb[:], mask_d[:])

            # ---- big persistent SBUF tensors ----
            xT = bigs.tile([P, ND, T], bf16)       # x transposed, [d_in_chunk, dc, t]
            qT_sb = bigs.tile([H, T], bf16)
            kT_sb = bigs.tile([H, T], bf16)
            vT_sb = bigs.tile([H, T], bf16)
            # one tile per T-block (not a single [P, NT, H+1] tensor): the
            # DMA-xbar transpose dest needs 32-byte SBUF alignment, which a
            # 65-element row stride would break for odd t
            v_sb = [
                bigs.tile([P, H + 1], bf16, name=f"v_sb{t}") for t in range(NT)
            ]
            probsT = bigs.tile([P, NT, T], bf16)    # exp'd transposed scores

            # ---- load + transpose x ----
            for t in range(NT):
                xb = xstage.tile([P, D], bf16, tag="xb")
                nc.gpsimd.dma_start(xb[:], x_d[ts(t, P), :])  # cast f32->bf16
                for dc in range(ND):
                    nc.sync.dma_start(
                        xT[:, dc, ts(t, P)], xb[:, ts(dc, P)], transpose=True
                    )

            # ---- projections qT/kT/vT = W.T @ x.T (accumulate over D-chunks) ----
            CW = 512
            NC_CHUNKS = T // CW  # 4
            with tc.tile_pool(name="psum_proj", bufs=2, space="PSUM") as psum_proj:
                for c in range(NC_CHUNKS):
                    psq = psum_proj.tile([H, CW], f32, tag="psq")
                    psk = psum_proj.tile([H, CW], f32, tag="psk")
                    psv = psum_proj.tile([H, CW], f32, tag="psv")
                    for dc in range(ND):
                        st = dc == 0
                        sp = dc == ND - 1
                        nc.tensor.matmul(
                            psq[:], wq_sb[:, dc, :], xT[:, dc, ts(c, CW)],
                            start=st, stop=sp,
                        )
                        nc.tensor.matmul(
                            psk[:], wk_sb[:, dc, :], xT[:, dc, ts(c, CW)],
                            start=st, stop=sp,
                        )
                        nc.tensor.matmul(
                            psv[:], wv_sb[:, dc, :], xT[:, dc, ts(c, CW)],
                            start=st, stop=sp,
                        )
                    nc.vector.tensor_copy(qT_sb[:, ts(c, CW)], psq[:])
                    nc.vector.tensor_copy(kT_sb[:, ts(c, CW)], psk[:])
                    nc.scalar.copy(vT_sb[:, ts(c, CW)], psv[:])

            # ---- v tiles [128, H] + ones column ----
            for t in range(NT):
                nc.sync.dma_start(v_sb[t][:, 0:H], vT_sb[:, ts(t, P)], transpose=True)
                nc.vector.memset(v_sb[t][:, H : H + 1], 1.0)

            # ---- attention: transposed scores -> exp -> (mask) -> PV ----
            psum_sT = tc.alloc_tile_pool(name="psum_sT", bufs=2, space="PSUM")
            psum_out = tc.alloc_tile_pool(name="psum_out", bufs=2, space="PSUM")

            def emit_scores(j):
                # sT[k in block j, q in [128j, T)] ; exp into probsT[:, j, :]
                q0 = P * j
                lq = T - q0
                off = 0
                while off < lq:
                    lc = min(SCORE_CHUNK, lq - off)
                    sT = psum_sT.tile([P, SCORE_CHUNK], f32, tag="sT")
                    sub = 0
                    while sub < lc:
                        w = min(512, lc - sub)
                        nc.tensor.matmul(
                            sT[:, sub : sub + w],
                            kT_sb[:, ts(j, P)],
                            qT_sb[:, q0 + off + sub : q0 + off + sub + w],
                            start=True,
                            stop=True,
                        )
                        sub += w
                    nc.scalar.activation(
                        probsT[:, j, q0 + off : q0 + off + lc],
                        sT[:, 0:lc],
                        Exp,
                        scale=SCALE,
                    )
                    off += lc
                # causal mask on the diagonal block (after exp: multiply by 0/1)
                nc.vector.tensor_mul(
                    probsT[:, j, q0 : q0 + P],
                    probsT[:, j, q0 : q0 + P],
                    mask_sb[:],
                )

            def emit_pv(qi):
                pso = psum_out.tile([P, H + 1], f32, tag="pso")
                # diagonal block first (start=True clears PSUM), then the rest
                order = [qi] + list(range(qi))
                for idx, kj in enumerate(order):
                    nc.tensor.matmul(
                        pso[:],
                        probsT[:, kj, ts(qi, P)],
                        v_sb[kj][:],
                        start=(idx == 0),
                        stop=(idx == len(order) - 1),
                    )
                rs = evac.tile([P, 1], f32, tag="rs")
                nc.vector.reciprocal(rs[:], pso[:, H : H + 1])
                ob = evac.tile([P, H], f32, tag="ob")
                nc.vector.tensor_scalar_mul(ob[:], pso[:, 0:H], rs[:])
                nc.sync.dma_start(out_d[ts(qi, P), :], ob[:])

            # software-pipeline by one iteration: PV(j-1) overlaps scores(j)
            for j in range(NT):
                emit_scores(j)
                if j >= 1:
                    emit_pv(j - 1)
            emit_pv(NT - 1)
            psum_out.release()
            psum_sT.release()

    nc.finalize()
    return nc


def _get_nc():
    if "nc" not in _CACHE:
        _CACHE["nc"] = _build_nc()
    return _CACHE["nc"]


def kernel(x, Wq, Wk, Wv):
    import ml_dtypes
    from concourse.bass_utils import run_bass_kernel_spmd

    x = np.asarray(x, dtype=np.float32)
    Wq = np.asarray(Wq, dtype=np.float32)
    Wk = np.asarray(Wk, dtype=np.float32)
    Wv = np.asarray(Wv, dtype=np.float32)

    # mask[k, q] = 1.0 where q >= k (upper-tri incl diagonal, sT layout)
    mask = np.triu(np.ones((P, P), dtype=np.float32)).astype(ml_dtypes.bfloat16)

    nc = _get_nc()
    in_maps = [
        {"x": x[b], "wq": Wq, "wk": Wk, "wv": Wv, "mask": mask}
        for b in range(NCORES)
    ]
    res = run_bass_kernel_spmd(nc, in_maps, core_ids=list(range(NCORES)))
    out = np.stack([np.asarray(res.results[b]["out"]) for b in range(NCORES)])
    return out.astype(np.float32)
